# revision 30
# baseline (speedup 1.0000x reference)
"""CRF marginal kernel for Trainium2 (8 NeuronCores, SPMD data-parallel over batch).

Reference math (keras_contrib CRF get_marginal_prob):
  e = X @ W + bias  (+ left/right boundary at t=0 / t=T-1)
  alpha/beta: logsumexp scans over T with transition chain[i,j]
  out = softmax_j(-(alpha_sr + e + beta_sl))

Kernel v2 (per core, B_local=8), all-fp16 datapath (validated in numsim.py:
rel err 8e-4 vs 2e-2 gate):
  - X is transposed + fp16-cast + stream-ordered on the HOST: xt[slice][p][c,j]
    with d on partitions, so the energy matmul needs NO on-chip transposes and
    half the HBM bytes. 8 slices of 2MB; each slice's 512 (t,b) columns are
    exactly the stripes the recurrence consumes at 4 consecutive steps.
  - Energy: per slice 16 accumulating fp16 matmuls [128,512] -> PSUM, then ACT
    exp evictions into QBUF (fp16, Q=exp(-e)) and Q3BUF (fp32, exp(-3e)) in
    scattered stripe order. Boundary bias variants at t=0/t=T-1.
  - Recurrence: linear-domain with constant rescale folded into
    EW[i,j]=exp(-chain[i,j]-CSCALE): v_{k+1} = EW^T (v_k*Q_k). 2 dirs x 8 segs
    x 8 batch = one [128,128] fp16 tile per step; NSTEP=80 (BURN=16 + 64).
    DVE multiply (fp32 PSUM state x fp16 Q -> fp16 qstore) + fp16 PE matmul.
  - Combine per 16-wide t-block in LOG space (no elementwise reciprocal --
    divide/approx-recip don't compile on this toolchain): margin =
    m3 - ln(qf*qb) with m3 = -3e stored fp32 during phase A; product, ACT
    Ln, subtract, PE transpose, ACT evict -> fp32 margins to DRAM. The
    softmax normalization happens on the HOST (0.02% of module FLOPs),
    removing Exp/row-sum/reciprocal/scale from the device tail. Product+
    subtract go to Pool for blocks that overlap the chain, DVE for the
    post-chain wave. Pumped/ring-buffered so each instruction carries at
    most one cross-engine sync wait (walrus limit).
"""

import numpy as np

B, T, D, F = 64, 512, 2048, 128
NCORES = 8
BL = B // NCORES  # 8 batch per core
H = 8  # segments per scan direction
SEG = T // H  # 64
BURN = 16  # burn-in steps per segment
NSTEP = SEG + BURN  # 80: muls k=0..79, matmuls k=0..78
NSLICE = 8
SCOLS = 512  # (t,b) columns per slice
PAD = BURN * BL  # 128 pad cols each side of QBUF
CSCALE = 5.3513  # mean per-step log-drift


def _slice_ks(i):
    return [4 * i + dk for dk in range(4)] if i < 4 else [16 + 4 * i + dk for dk in range(4)]


def _sF(k):  # fwd stripe consumed at step k
    return 48 + k if k < 16 else k - 16


def _sB(k):  # bwd stripe consumed at step k
    return 15 - k if k < 16 else 79 - k


def build_nc():
    import concourse.bass as bass
    import concourse.mybir as mybir
    from concourse.tile import TileContext
    from concourse.ap import AP

    fp32 = mybir.dt.float32
    fp16 = mybir.dt.float16
    Act = mybir.ActivationFunctionType
    Alu = mybir.AluOpType

    nc = bass.Bass()
    XTd = nc.declare_dram_parameter("xt", [NSLICE, 128, 16 * SCOLS], fp16, isOutput=False)
    Wd = nc.declare_dram_parameter("w", [128, 16 * 128], fp16, isOutput=False)
    EWd = nc.declare_dram_parameter("ew", [F, F], fp16, isOutput=False)
    NBd = nc.declare_dram_parameter("nb", [F, 6], fp32, isOutput=False)
    IDd = nc.declare_dram_parameter("idn", [F, F], fp32, isOutput=False)
    OUTd = nc.declare_dram_parameter("out", [BL, T, F], fp32, isOutput=True)

    def sub(base, col_off, dims):
        return AP(
            tensor=base.tensor,
            offset=base.offset + col_off,
            ap=[list(base.ap[0])] + [list(d) for d in dims],
        )

    def part(ap, n, dims=None):
        """Partition-sliced view (first n partitions)."""
        rest = [list(d) for d in (dims if dims is not None else ap.ap[1:])]
        return AP(tensor=ap.tensor, offset=ap.offset,
                  ap=[[list(ap.ap[0])[0], n]] + rest)

    def qcol(t):  # QBUF column of (t, b=0)
        return PAD + t * BL

    def pump(ap):
        """PE observation pump: 1-col ldweights with a genuine cross-engine
        data dep, absorbing one producer's wait so the matmul that follows
        carries at most one."""
        if ap.dtype != fp16:
            ap = ap.bitcast(fp16)
        nc.tensor.ldweights(ap)

    with TileContext(nc) as tc:
        with (
            tc.tile_pool(name="const", bufs=1) as constp,
            tc.tile_pool(name="big", bufs=1) as bigp,
            tc.tile_pool(name="xtp", bufs=3) as xtp,
            tc.tile_pool(name="pep", bufs=2, space="PSUM") as pep,
            tc.tile_pool(name="statef", bufs=2, space="PSUM") as statef,
            tc.tile_pool(name="stateb", bufs=2, space="PSUM") as stateb,
            tc.tile_pool(name="pup", bufs=2, space="PSUM") as pup,
        ):
            # ---- constants ----
            w16 = constp.tile([128, 16 * 128], fp16, name="w16")
            nc.sync.dma_start(out=w16[:], in_=Wd[:])
            ew16 = constp.tile([128, 128], fp16, name="ew16")
            nc.sync.dma_start(out=ew16[:], in_=EWd[:])
            nb_sb = constp.tile([128, 6], fp32, name="nb_sb")
            nc.sync.dma_start(out=nb_sb[:], in_=NBd[:])
            id_ld = constp.tile([128, 128], fp32, name="id_ld")
            nc.sync.dma_start(out=id_ld[:], in_=IDd[:])
            id_sb = constp.tile([128, 128], fp32, name="id_sb")
            nc.vector.tensor_copy(id_sb[:], id_ld[:])

            # ---- persistent big buffers ----
            qbuf = bigp.tile([128, 2 * PAD + T * BL], fp16, name="qbuf")
            nc.vector.memset(qbuf[:, :PAD], 1.0)
            nc.vector.memset(qbuf[:, PAD + T * BL:], 1.0)
            m3buf = bigp.tile([128, T * BL], fp32, name="m3buf")
            qstore = bigp.tile([128, NSTEP * 128], fp16, name="qstore")
            scrq = bigp.tile([128, 2 * NSTEP], fp16, name="scrq")
            scrs = bigp.tile([128, 2 * NSTEP], fp32, name="scrs")
            mbring = bigp.tile([128, 32 * 128], fp32, name="mbring")
            lmring = bigp.tile([128, 32 * 128], fp32, name="lmring")
            mgnring = bigp.tile([128, 32 * 128], fp32, name="mgnring")
            obring = bigp.tile([128, 32 * 128], fp32, name="obring")
            scrap = bigp.tile([128, 2], fp32, name="scrap")

            # ACT warmup: an ACT-engine read of nb_sb so every later ACT
            # instruction is ordered after the nb DMA (keeps evictions at
            # one sync wait).
            nc.scalar.activation(scrap[:, 0:1], nb_sb[:, 1:2], Act.Copy)
            # PE warmup: throwaway transpose absorbs the id_sb DVE-copy dep
            # so combine transposes carry a single wait.
            warm = pup.tile([128, 128], fp32, name="pu")
            nc.tensor.transpose(warm[:], id_sb[:], id_sb[:])

            # two independent half-chains (fwd / bwd) ping-pong on DVE+PE so
            # the serial sem latency of one hides the other's compute
            prev_ps = [None, None]

            def emit_half(k, h, pumps):
                off = qcol(k - BURN) if h == 0 else qcol(SEG + BURN - 1 - k)
                qin = sub(qbuf, off, [[SEG * BL, H], [1, BL]])
                qout = sub(qstore, k * 128 + h * 64, [[8, H], [1, BL]])
                if pumps:
                    # DVE pump: the coalesced ACT wait lands here, not on the mul
                    nc.vector.tensor_copy(
                        sub(scrq, 2 * k + h, [[1, 1], [1, 1]]),
                        sub(qbuf, off, [[1, 1], [1, 1]]))
                if k == 0:
                    nc.vector.tensor_copy(qout, qin)
                else:
                    if pumps:
                        # DVE pump: absorb the PSUM-state (PE) wait
                        nc.vector.tensor_copy(
                            scrs[:, 2 * k + h:2 * k + h + 1],
                            sub(prev_ps[h], 0, [[1, 1]]))
                    pin = sub(prev_ps[h], 0, [[8, H], [1, BL]])
                    nc.vector.tensor_tensor(qout, pin, qin, op=Alu.mult)
                if k == BURN:
                    # exact init: fwd seg0 q = Q_{t=0}, bwd blk7 q = Q_{T-1}
                    t_ow = 0 if h == 0 else T - 1
                    c_ow = 0 if h == 0 else 120
                    nc.vector.tensor_copy(
                        sub(qstore, k * 128 + c_ow, [[1, 1], [1, BL]]),
                        sub(qbuf, qcol(t_ow), [[1, 1], [1, BL]]))
                if k < NSTEP - 1:
                    st = (statef if h == 0 else stateb).tile([128, 64], fp32, name="st")
                    nc.tensor.matmul(
                        st[:], ew16[:],
                        sub(qstore, k * 128 + h * 64, [[1, 64]]),
                        start=True, stop=True,
                    )
                    prev_ps[h] = st

            def emit_step(k, pumps=True):
                emit_half(k, 0, pumps)
                emit_half(k, 1, pumps)

            def emit_slice(i):
                ks = _slice_ks(i)
                xt = xtp.tile([128, 16 * SCOLS], fp16, name="xt")
                # 4 quarter-DMAs so the first matmuls start ~3.5us earlier
                for p in range(4):
                    xin = AP(tensor=XTd,
                             offset=i * 128 * 16 * SCOLS + p * 4 * SCOLS,
                             ap=[[16 * SCOLS, 128], [1, 4 * SCOLS]])
                    nc.sync.dma_start(
                        out=xt[:, p * 4 * SCOLS:(p + 1) * 4 * SCOLS], in_=xin)
                pe = pep.tile([128, SCOLS], fp32, name="pe")
                for c in range(16):
                    if c % 4 == 0:
                        pump(xt[:, c * SCOLS:c * SCOLS + 2])
                    nc.tensor.matmul(
                        pe[:], w16[:, c * 128:(c + 1) * 128],
                        xt[:, c * SCOLS:(c + 1) * SCOLS],
                        start=(c == 0), stop=(c == 15),
                    )
                # evictions: Q (fp16, exp(-e)) then m3 (fp32, -3e)
                # psum col = dir*256 + dk*64 + g*8 + b
                # qbuf col = PAD + (g*64 + s)*8 + b,  s = s0 +/- dk
                for m3 in (False, True):
                    dst = m3buf if m3 else qbuf
                    base_pad = 0 if m3 else PAD
                    func = Act.Identity if m3 else Act.Exp
                    scale = -3.0 if m3 else -1.0
                    bc_main = 4 if m3 else 1
                    bc_left = 3 if m3 else 0
                    bc_right = 5 if m3 else 2
                    for d in range(2):
                        s0 = _sF(ks[0]) if d == 0 else _sB(ks[0])
                        sgn = 1 if d == 0 else -1
                        if i != 3:
                            pin = sub(pe, d * 256, [[64, 4], [8, 8], [1, 8]])
                            qo = sub(dst, base_pad + s0 * 8,
                                     [[sgn * 8, 4], [512, 8], [1, 8]])
                            nc.scalar.activation(
                                qo, pin, func,
                                bias=nb_sb[:, bc_main:bc_main + 1], scale=scale)
                        else:
                            # dk 0..2 full
                            pin = sub(pe, d * 256, [[64, 3], [8, 8], [1, 8]])
                            qo = sub(dst, base_pad + s0 * 8,
                                     [[sgn * 8, 3], [512, 8], [1, 8]])
                            nc.scalar.activation(
                                qo, pin, func,
                                bias=nb_sb[:, bc_main:bc_main + 1], scale=scale)
                            s3 = s0 + 3 * sgn  # 63 (fwd) or 0 (bwd)
                            if d == 0:
                                # dk=3: g=0..6 normal, g=7 is t=511
                                pin = sub(pe, 192, [[8, 7], [1, 8]])
                                qo = sub(dst, base_pad + s3 * 8, [[512, 7], [1, 8]])
                                nc.scalar.activation(
                                    qo, pin, func,
                                    bias=nb_sb[:, bc_main:bc_main + 1], scale=scale)
                                pin = sub(pe, 192 + 56, [[1, 8]])
                                qo = sub(dst, base_pad + 511 * 8, [[1, 8]])
                                nc.scalar.activation(
                                    qo, pin, func,
                                    bias=nb_sb[:, bc_right:bc_right + 1], scale=scale)
                            else:
                                # dk=3: g=1..7 normal, g=0 is t=0
                                pin = sub(pe, 256 + 192 + 8, [[8, 7], [1, 8]])
                                qo = sub(dst, base_pad + s3 * 8 + 512, [[512, 7], [1, 8]])
                                nc.scalar.activation(
                                    qo, pin, func,
                                    bias=nb_sb[:, bc_main:bc_main + 1], scale=scale)
                                pin = sub(pe, 256 + 192, [[1, 8]])
                                qo = sub(dst, base_pad + 0, [[1, 8]])
                                nc.scalar.activation(
                                    qo, pin, func,
                                    bias=nb_sb[:, bc_left:bc_left + 1], scale=scale)

            def emit_combine_p1(t0, on_pool):
                bi = t0 // 16
                g, r = t0 // SEG, t0 % SEG
                kf0 = r + BURN
                kb0 = SEG + BURN - 1 - r
                qf = sub(qstore, kf0 * 128 + g * 8, [[1, BL], [128, 16]])
                qb = sub(qstore, kb0 * 128 + 64 + g * 8, [[1, BL], [-128, 16]])
                e1 = nc.gpsimd if on_pool else nc.vector
                mb = mbring[:, bi * 128:(bi + 1) * 128]
                e1.tensor_tensor(mb, qf, qb, op=Alu.mult)
                lm = lmring[:, bi * 128:(bi + 1) * 128]
                nc.scalar.activation(lm, mb, Act.Ln)
                m3a = sub(m3buf, t0 * BL, [[1, BL], [8, 16]])
                mgn = mgnring[:, bi * 128:(bi + 1) * 128]
                e1.tensor_tensor(mgn, m3a, lm, op=Alu.subtract)

            def emit_combine_p2(t0):
                bi = t0 // 16
                mgn = mgnring[:, bi * 128:(bi + 1) * 128]
                pu = pup.tile([128, 128], fp32, name="pu")
                pump(mgn[:, 0:2])
                nc.tensor.transpose(pu[:], mgn, id_sb[:])
                ob = obring[:, bi * 128:(bi + 1) * 128]
                nc.scalar.activation(ob, pu[:], Act.Copy)
                # two half-DMAs (b 0-3 / 4-7), alternating dispatch queues
                eng = (nc.sync, nc.gpsimd)[bi % 2]
                for hb in range(2):
                    oap = AP(tensor=OUTd, offset=hb * 4 * T * F + t0 * F,
                             ap=[[T * F, 4], [F, 16], [1, F]])
                    src = AP(tensor=ob.tensor, offset=ob.offset + hb * 64 * ob.ap[0][0],
                             ap=[[ob.ap[0][0], 64], [1, 128]])
                    eng.dma_start(out=oap, in_=src)

            steps_after = [
                list(range(0, 4)), list(range(4, 8)), list(range(8, 12)),
                list(range(12, 24)), list(range(24, 36)), list(range(36, 40)),
                list(range(40, 44)), list(range(44, 48)),
            ]
            for i in range(NSLICE):
                emit_slice(i)
                # keep each slice's 16 energy matmuls contiguous on the PE
                # queue (interleaved chain matmuls break the PE sequencer's
                # weight-load overlap, halving matmul rate)
                tc.no_sync_barrier()
                for k in steps_after[i]:
                    emit_step(k)

            ready = {}
            for t0 in range(0, T, 16):
                r = t0 % SEG
                ready.setdefault(max(r + BURN + 15, SEG + BURN - 1 - r), []).append(t0)
            p1q, p2q = [], []
            for k in range(48, NSTEP):
                emit_step(k, pumps=(k <= 48))
                p1q.extend(ready.get(k, []))
                if k < NSTEP - 1:
                    # pace part-1 (Pool+ACT only, off the chain's DVE/PE path)
                    # 2/step; part-2 (PE transposes) strictly post-chain
                    for _ in range(min(2, len(p1q))):
                        t0 = p1q.pop(0)
                        emit_combine_p1(t0, on_pool=True)
                        p2q.append(t0)
            # scheduler fence: keep the tail's DVE/PE work out of the
            # latency-critical chain (no runtime sync cost)
            tc.no_sync_barrier()
            # post-chain: remaining part-1 on DVE (idle now), then part-2
            for t0 in p1q:
                emit_combine_p1(t0, on_pool=False)
                p2q.append(t0)
            for t0 in p2q:
                emit_combine_p2(t0)

    _strip_waits(nc)
    return nc


def _strip_waits(nc):
    """Reduce every instruction to <=1 sync wait (walrus limit), using only
    drops that hardware ordering or this kernel's structure guarantees:
    - duplicate-sem waits merged to the max value (always sound);
    - PE->PE waits on PE instructions: PE completions are pc-monotone;
    - ACT evictions' DVE waits: bounding-box WAR artifacts vs chain muls
      reading strictly different QBUF/Q3 stripe columns;
    - DMA-DMA waits on output DMAs: disjoint OUT regions (box artifacts);
    - kernel-tail drains keep the out-DMA wait.
    """
    import concourse.mybir as mybir

    own_sem = {"Pool": "Pool_", "DVE": "DVE_", "Activation": "Activation_",
               "PE": "PE_"}
    warn = []
    for f in nc.m.functions:
        for bb in f.blocks:
            for inst in bb.instructions:
                si = inst.sync_info
                if si is None or len(si.on_wait) == 0:
                    continue
                tn = type(inst).__name__
                eng = str(inst.engine).split(".")[-1]
                if len(si.on_wait) == 1:
                    # single-wait fast path: only the Pool-on-DVE artifact
                    # drop applies (see below)
                    if (eng == "DVE" and tn != "InstDMACopy"
                            and si.on_wait[0].ant_name.startswith("Pool_")):
                        inst.sync_info = mybir.SyncInfo(
                            on_wait=[], on_update=list(si.on_update))
                    continue
                best = {}
                for x in si.on_wait:
                    if x.ant_name not in best or x.wait_value > best[x.ant_name].wait_value:
                        best[x.ant_name] = x
                w = list(best.values())

                def setw(w2):
                    inst.sync_info = mybir.SyncInfo(
                        on_wait=w2, on_update=list(si.on_update))

                # own-engine sem waits: engines execute in issue order
                own = own_sem.get(eng)
                if own and len(w) > 1 and tn != "InstDMACopy":
                    w = [x for x in w if not x.ant_name.startswith(own)] or w[:1]
                # no DVE instruction in this kernel reads Pool-written data:
                # Pool->DVE waits are qstore bounding-box WAR artifacts
                # (combine reads of old steps vs chain writes of new steps)
                if eng == "DVE" and tn != "InstDMACopy":
                    w = [x for x in w if not x.ant_name.startswith("Pool_")]
                if len(w) <= 1:
                    setw(w)
                    continue
                if tn in ("InstMatmult", "InstLdweights"):
                    w = [x for x in w if not x.ant_name.startswith("PE_")]
                elif tn == "InstActivation":
                    pe = [x for x in w if x.ant_name.startswith("PE_")]
                    if pe:
                        w = pe
                elif tn == "InstDMACopy":
                    w = [x for x in w if not (
                        x.ant_name.startswith("DMASW")
                        or x.ant_name.startswith("DMAHW"))]
                elif tn == "InstDrain":
                    w.sort(key=lambda x: 0 if x.ant_name.startswith("DMA") else 1)
                    w = w[:1]
                if len(w) > 1:
                    warn.append((tn, str(inst.engine), [x.ant_name for x in w]))
                    rank = {"PE": 0, "Ac": 1, "DV": 2, "Po": 3}
                    w.sort(key=lambda x: rank.get(x.ant_name[:2], 4))
                    w = w[:1]
                setw(w)
    if warn:
        from collections import Counter
        cnt = Counter((t, e, tuple(ws)) for t, e, ws in warn)
        for k, v in cnt.items():
            print(f"WARN multi-wait fallback x{v}: {k}")


_SLICE_IDX = None


def _slice_indices():
    global _SLICE_IDX
    if _SLICE_IDX is not None:
        return _SLICE_IDX
    out = []
    for i in range(NSLICE):
        ks = _slice_ks(i)
        ts = np.zeros(SCOLS, np.int64)
        bs = np.zeros(SCOLS, np.int64)
        for d in range(2):
            for dk in range(4):
                k = ks[dk]
                s = _sF(k) if d == 0 else _sB(k)
                for g in range(8):
                    for b in range(8):
                        j = d * 256 + dk * 64 + g * 8 + b
                        ts[j] = g * SEG + s
                        bs[j] = b
        out.append((ts, bs))
    _SLICE_IDX = out
    return out


def host_inputs(X, kernel, chain_kernel, bias, left_boundary, right_boundary):
    X = np.asarray(X, np.float32)
    W = np.asarray(kernel, np.float32)
    C = np.asarray(chain_kernel, np.float32)
    bias = np.asarray(bias, np.float32)
    lb = np.asarray(left_boundary, np.float32)
    rb = np.asarray(right_boundary, np.float32)

    EW16 = np.exp(-C.astype(np.float64) - CSCALE).astype(np.float16)
    W16 = np.ascontiguousarray(
        W.astype(np.float16).reshape(16, 128, 128).transpose(1, 0, 2)
    ).reshape(128, 16 * 128)
    nb0, nb1, nb2 = -(bias + lb), -bias, -(bias + rb)
    NB = np.stack([nb0, nb1, nb2, 3 * nb0, 3 * nb1, 3 * nb2], axis=1).astype(np.float32)
    IDN = np.eye(F, dtype=np.float32)

    X16 = X.astype(np.float16)
    idx = _slice_indices()
    in_maps = []
    for c in range(NCORES):
        Xc = X16[c * BL:(c + 1) * BL]  # (8, 512, 2048)
        xts = np.empty((NSLICE, 128, 16 * SCOLS), np.float16)
        for i in range(NSLICE):
            ts, bs = idx[i]
            cols = Xc[bs, ts, :]  # (512, 2048)
            xts[i] = np.ascontiguousarray(
                cols.T.reshape(16, 128, SCOLS).transpose(1, 0, 2)
            ).reshape(128, 16 * SCOLS)
        in_maps.append({
            "xt": xts, "w": W16, "ew": EW16, "nb": NB, "idn": IDN,
        })
    return in_maps


_NC_CACHE = None


def kernel(X, kernel, chain_kernel, bias, left_boundary, right_boundary):
    global _NC_CACHE
    from concourse.bass_utils import run_bass_kernel_spmd

    if _NC_CACHE is None:
        _NC_CACHE = build_nc()
    nc = _NC_CACHE
    in_maps = host_inputs(X, kernel, chain_kernel, bias, left_boundary, right_boundary)
    res = run_bass_kernel_spmd(nc, in_maps, list(range(NCORES)))
    return postprocess(res)


def postprocess(res):
    # device returns pre-softmax margins (fp32); normalize on host
    m = np.concatenate(
        [np.asarray(res.results[c]["out"], np.float32) for c in range(NCORES)],
        axis=0)
    m -= m.max(-1, keepdims=True)
    np.exp(m, out=m)
    m /= m.sum(-1, keepdims=True)
    return m


# revision 33
# speedup vs baseline: 1.1724x; 1.1724x over previous
"""CRF marginal kernel for Trainium2 (8 NeuronCores, SPMD data-parallel over batch).

Reference math (keras_contrib CRF get_marginal_prob):
  e = X @ W + bias  (+ left/right boundary at t=0 / t=T-1)
  alpha/beta: logsumexp scans over T with transition chain[i,j]
  out = softmax_j(-(alpha_sr + e + beta_sl))

Kernel v2 (per core, B_local=8), all-fp16 datapath (validated in numsim.py:
rel err 8e-4 vs 2e-2 gate):
  - X is transposed + fp16-cast + stream-ordered on the HOST: xt[slice][p][c,j]
    with d on partitions, so the energy matmul needs NO on-chip transposes and
    half the HBM bytes. 8 slices of 2MB; each slice's 512 (t,b) columns are
    exactly the stripes the recurrence consumes at 4 consecutive steps.
  - Energy: per slice 16 accumulating fp16 matmuls [128,512] -> PSUM, then ACT
    exp evictions into QBUF (fp16, Q=exp(-e)) and Q3BUF (fp32, exp(-3e)) in
    scattered stripe order. Boundary bias variants at t=0/t=T-1.
  - Recurrence: linear-domain with constant rescale folded into
    EW[i,j]=exp(-chain[i,j]-CSCALE): v_{k+1} = EW^T (v_k*Q_k). 2 dirs x 8 segs
    x 8 batch = one [128,128] fp16 tile per step; NSTEP=80 (BURN=16 + 64).
    DVE multiply (fp32 PSUM state x fp16 Q -> fp16 qstore) + fp16 PE matmul.
  - Combine per 16-wide t-block in LOG space (no elementwise reciprocal --
    divide/approx-recip don't compile on this toolchain): margin =
    m3 - ln(qf*qb) with m3 = -3e stored fp32 during phase A; product, ACT
    Ln, subtract, PE transpose, ACT evict -> fp32 margins to DRAM. The
    softmax normalization happens on the HOST (0.02% of module FLOPs),
    removing Exp/row-sum/reciprocal/scale from the device tail. Product+
    subtract go to Pool for blocks that overlap the chain, DVE for the
    post-chain wave. Pumped/ring-buffered so each instruction carries at
    most one cross-engine sync wait (walrus limit).
"""

import numpy as np

B, T, D, F = 64, 512, 2048, 128
NCORES = 8
BL = B // NCORES  # 8 batch per core
H = 8  # segments per scan direction
SEG = T // H  # 64
BURN = 16  # burn-in steps per segment
NSTEP = SEG + BURN  # 80: muls k=0..79, matmuls k=0..78
NSLICE = 8
SCOLS = 512  # (t,b) columns per slice
QS = 72  # QBUF cols per stripe: 64 data + shared 8-col one-pads between
CSCALE = 5.3513  # mean per-step log-drift


def _slice_ks(i):
    return [4 * i + dk for dk in range(4)] if i < 4 else [16 + 4 * i + dk for dk in range(4)]


def _sF(k):  # fwd stripe consumed at step k
    return 48 + k if k < 16 else k - 16


def _sB(k):  # bwd stripe consumed at step k
    return 15 - k if k < 16 else 79 - k


def build_nc():
    import concourse.bass as bass
    import concourse.mybir as mybir
    from concourse.tile import TileContext
    from concourse.ap import AP

    fp32 = mybir.dt.float32
    fp16 = mybir.dt.float16
    Act = mybir.ActivationFunctionType
    Alu = mybir.AluOpType

    nc = bass.Bass()
    XTd = nc.declare_dram_parameter("xt", [NSLICE, 128, 16 * SCOLS], fp16, isOutput=False)
    Wd = nc.declare_dram_parameter("w", [128, 16 * 128], fp16, isOutput=False)
    EWd = nc.declare_dram_parameter("ew", [F, F], fp16, isOutput=False)
    NBd = nc.declare_dram_parameter("nb", [F, 6], fp32, isOutput=False)
    IDd = nc.declare_dram_parameter("idn", [F, F], fp32, isOutput=False)
    OUTd = nc.declare_dram_parameter("out", [BL, T, F], fp32, isOutput=True)

    def sub(base, col_off, dims):
        return AP(
            tensor=base.tensor,
            offset=base.offset + col_off,
            ap=[list(base.ap[0])] + [list(d) for d in dims],
        )

    def part(ap, n, dims=None):
        """Partition-sliced view (first n partitions)."""
        rest = [list(d) for d in (dims if dims is not None else ap.ap[1:])]
        return AP(tensor=ap.tensor, offset=ap.offset,
                  ap=[[list(ap.ap[0])[0], n]] + rest)

    def qdata(s):  # QBUF column of stripe s, (g=0, b=0)
        return 8 + s * QS

    def pump(ap):
        """PE observation pump: 1-col ldweights with a genuine cross-engine
        data dep, absorbing one producer's wait so the matmul that follows
        carries at most one."""
        if ap.dtype != fp16:
            ap = ap.bitcast(fp16)
        nc.tensor.ldweights(ap)

    with TileContext(nc) as tc:
        with (
            tc.tile_pool(name="const", bufs=1) as constp,
            tc.tile_pool(name="big", bufs=1) as bigp,
            tc.tile_pool(name="xtp", bufs=3) as xtp,
            tc.tile_pool(name="pep", bufs=2, space="PSUM") as pep,
            tc.tile_pool(name="statef", bufs=2, space="PSUM") as statef,
            tc.tile_pool(name="stateb", bufs=2, space="PSUM") as stateb,
            tc.tile_pool(name="pup", bufs=2, space="PSUM") as pup,
        ):
            # ---- constants ----
            w16 = constp.tile([128, 16 * 128], fp16, name="w16")
            nc.sync.dma_start(out=w16[:], in_=Wd[:])
            ew16 = constp.tile([128, 128], fp16, name="ew16")
            nc.sync.dma_start(out=ew16[:], in_=EWd[:])
            nb_sb = constp.tile([128, 6], fp32, name="nb_sb")
            nc.sync.dma_start(out=nb_sb[:], in_=NBd[:])
            id_ld = constp.tile([128, 128], fp32, name="id_ld")
            nc.sync.dma_start(out=id_ld[:], in_=IDd[:])
            id_sb = constp.tile([128, 128], fp32, name="id_sb")
            nc.vector.tensor_copy(id_sb[:], id_ld[:])

            # ---- persistent big buffers ----
            # stripe-major: [8 pad][64 data s][8 pad][64 data s+1]... pads
            # stay 1.0 (memset), evictions fill the data blocks contiguously
            qbuf = bigp.tile([128, 8 + 64 * QS], fp16, name="qbuf")
            nc.vector.memset(qbuf[:], 1.0)
            m3buf = bigp.tile([128, T * BL], fp32, name="m3buf")
            qstore = bigp.tile([128, NSTEP * 128], fp16, name="qstore")
            scrq = bigp.tile([128, 2 * NSTEP], fp16, name="scrq")
            scrs = bigp.tile([128, 2 * NSTEP], fp32, name="scrs")
            mbring = bigp.tile([128, 32 * 128], fp32, name="mbring")
            lmring = bigp.tile([128, 32 * 128], fp32, name="lmring")
            mgnring = bigp.tile([128, 32 * 128], fp32, name="mgnring")
            obring = bigp.tile([128, 32 * 128], fp32, name="obring")
            scrap = bigp.tile([128, 2], fp32, name="scrap")

            # ACT warmup: an ACT-engine read of nb_sb so every later ACT
            # instruction is ordered after the nb DMA (keeps evictions at
            # one sync wait).
            nc.scalar.activation(scrap[:, 0:1], nb_sb[:, 1:2], Act.Copy)
            # PE warmup: throwaway transpose absorbs the id_sb DVE-copy dep
            # so combine transposes carry a single wait.
            warm = pup.tile([128, 128], fp32, name="pu")
            nc.tensor.transpose(warm[:], id_sb[:], id_sb[:])

            # two independent half-chains (fwd / bwd) ping-pong on DVE+PE so
            # the serial sem latency of one hides the other's compute
            prev_ps = [None, None]

            def emit_half(k, h, pumps):
                s = _sF(k) if h == 0 else _sB(k)
                shift = ((-8 if h == 0 else 8) if k < BURN else 0)
                off = qdata(s) + shift
                qin = sub(qbuf, off, [[8, H], [1, BL]])
                qout = sub(qstore, k * 128 + h * 64, [[8, H], [1, BL]])
                if pumps:
                    # DVE pump: the coalesced ACT wait lands here, not on the mul
                    nc.vector.tensor_copy(
                        sub(scrq, 2 * k + h, [[1, 1], [1, 1]]),
                        sub(qbuf, off, [[1, 1], [1, 1]]))
                if k == 0:
                    nc.vector.tensor_copy(qout, qin)
                else:
                    if pumps:
                        # DVE pump: absorb the PSUM-state (PE) wait
                        nc.vector.tensor_copy(
                            scrs[:, 2 * k + h:2 * k + h + 1],
                            sub(prev_ps[h], 0, [[1, 1]]))
                    pin = sub(prev_ps[h], 0, [[8, H], [1, BL]])
                    nc.vector.tensor_tensor(qout, pin, qin, op=Alu.mult)
                if k == BURN:
                    # exact init: fwd seg0 q = Q_{t=0}, bwd blk7 q = Q_{T-1}
                    q_ow = qdata(0) if h == 0 else qdata(63) + 56
                    c_ow = 0 if h == 0 else 120
                    nc.vector.tensor_copy(
                        sub(qstore, k * 128 + c_ow, [[1, 1], [1, BL]]),
                        sub(qbuf, q_ow, [[1, 1], [1, BL]]))
                if k < NSTEP - 1:
                    st = (statef if h == 0 else stateb).tile([128, 64], fp32, name="st")
                    nc.tensor.matmul(
                        st[:], ew16[:],
                        sub(qstore, k * 128 + h * 64, [[1, 64]]),
                        start=True, stop=True,
                    )
                    prev_ps[h] = st

            def emit_step(k, pumps=True):
                emit_half(k, 0, pumps)
                emit_half(k, 1, pumps)

            def emit_slice(i):
                ks = _slice_ks(i)
                xt = xtp.tile([128, 16 * SCOLS], fp16, name="xt")
                xin = AP(tensor=XTd, offset=i * 128 * 16 * SCOLS,
                         ap=[[16 * SCOLS, 128], [1, 16 * SCOLS]])
                nc.sync.dma_start(out=xt[:], in_=xin)
                pump(xt[:, 0:2])
                pe = pep.tile([128, SCOLS], fp32, name="pe")
                for c in range(16):
                    nc.tensor.matmul(
                        pe[:], w16[:, c * 128:(c + 1) * 128],
                        xt[:, c * SCOLS:(c + 1) * SCOLS],
                        start=(c == 0), stop=(c == 15),
                    )
                # evictions: Q (fp16, exp(-e)) then m3 (fp32, -3e)
                # psum col = dir*256 + dk*64 + g*8 + b
                # qbuf col = PAD + (g*64 + s)*8 + b,  s = s0 +/- dk
                for m3 in (False, True):
                    dst = m3buf if m3 else qbuf
                    st_w = 64 if m3 else QS  # stripe stride in dst
                    st_b = 0 if m3 else 8    # data base offset
                    func = Act.Identity if m3 else Act.Exp
                    scale = -3.0 if m3 else -1.0
                    bc_main = 4 if m3 else 1
                    bc_left = 3 if m3 else 0
                    bc_right = 5 if m3 else 2

                    def dcol(s, g=0):  # dst col of (stripe s, seg g, b=0)
                        return st_b + s * st_w + g * 8

                    for d in range(2):
                        s0 = _sF(ks[0]) if d == 0 else _sB(ks[0])
                        sgn = 1 if d == 0 else -1
                        if i != 3:
                            pin = sub(pe, d * 256, [[64, 4], [8, 8], [1, 8]])
                            qo = sub(dst, dcol(s0),
                                     [[sgn * st_w, 4], [8, 8], [1, 8]])
                            nc.scalar.activation(
                                qo, pin, func,
                                bias=nb_sb[:, bc_main:bc_main + 1], scale=scale)
                        else:
                            # dk 0..2 full
                            pin = sub(pe, d * 256, [[64, 3], [8, 8], [1, 8]])
                            qo = sub(dst, dcol(s0),
                                     [[sgn * st_w, 3], [8, 8], [1, 8]])
                            nc.scalar.activation(
                                qo, pin, func,
                                bias=nb_sb[:, bc_main:bc_main + 1], scale=scale)
                            s3 = s0 + 3 * sgn  # 63 (fwd) or 0 (bwd)
                            if d == 0:
                                # dk=3: g=0..6 normal, g=7 is t=511
                                pin = sub(pe, 192, [[8, 7], [1, 8]])
                                qo = sub(dst, dcol(s3), [[8, 7], [1, 8]])
                                nc.scalar.activation(
                                    qo, pin, func,
                                    bias=nb_sb[:, bc_main:bc_main + 1], scale=scale)
                                pin = sub(pe, 192 + 56, [[1, 8]])
                                qo = sub(dst, dcol(s3, 7), [[1, 8]])
                                nc.scalar.activation(
                                    qo, pin, func,
                                    bias=nb_sb[:, bc_right:bc_right + 1], scale=scale)
                            else:
                                # dk=3: g=1..7 normal, g=0 is t=0
                                pin = sub(pe, 256 + 192 + 8, [[8, 7], [1, 8]])
                                qo = sub(dst, dcol(s3, 1), [[8, 7], [1, 8]])
                                nc.scalar.activation(
                                    qo, pin, func,
                                    bias=nb_sb[:, bc_main:bc_main + 1], scale=scale)
                                pin = sub(pe, 256 + 192, [[1, 8]])
                                qo = sub(dst, dcol(s3, 0), [[1, 8]])
                                nc.scalar.activation(
                                    qo, pin, func,
                                    bias=nb_sb[:, bc_left:bc_left + 1], scale=scale)

            def emit_combine_p1(t0, on_pool):
                bi = t0 // 16
                g, r = t0 // SEG, t0 % SEG
                kf0 = r + BURN
                kb0 = SEG + BURN - 1 - r
                qf = sub(qstore, kf0 * 128 + g * 8, [[1, BL], [128, 16]])
                qb = sub(qstore, kb0 * 128 + 64 + g * 8, [[1, BL], [-128, 16]])
                e1 = nc.gpsimd if on_pool else nc.vector
                mb = mbring[:, bi * 128:(bi + 1) * 128]
                e1.tensor_tensor(mb, qf, qb, op=Alu.mult)
                lm = lmring[:, bi * 128:(bi + 1) * 128]
                nc.scalar.activation(lm, mb, Act.Ln)
                m3a = sub(m3buf, (t0 % SEG) * 64 + g * 8, [[1, BL], [64, 16]])
                mgn = mgnring[:, bi * 128:(bi + 1) * 128]
                e1.tensor_tensor(mgn, m3a, lm, op=Alu.subtract)

            def emit_combine_p2(t0):
                bi = t0 // 16
                mgn = mgnring[:, bi * 128:(bi + 1) * 128]
                pu = pup.tile([128, 128], fp32, name="pu")
                pump(mgn[:, 0:2])
                nc.tensor.transpose(pu[:], mgn, id_sb[:])
                ob = obring[:, bi * 128:(bi + 1) * 128]
                nc.scalar.activation(ob, pu[:], Act.Copy)
                # two half-DMAs (b 0-3 / 4-7), alternating dispatch queues
                eng = (nc.sync, nc.gpsimd)[bi % 2]
                for hb in range(2):
                    oap = AP(tensor=OUTd, offset=hb * 4 * T * F + t0 * F,
                             ap=[[T * F, 4], [F, 16], [1, F]])
                    src = AP(tensor=ob.tensor, offset=ob.offset + hb * 64 * ob.ap[0][0],
                             ap=[[ob.ap[0][0], 64], [1, 128]])
                    eng.dma_start(out=oap, in_=src)

            steps_after = [
                list(range(0, 4)), list(range(4, 8)), list(range(8, 12)),
                list(range(12, 24)), list(range(24, 36)), list(range(36, 40)),
                list(range(40, 44)), list(range(44, 48)),
            ]
            for i in range(NSLICE):
                emit_slice(i)
                for k in steps_after[i]:
                    emit_step(k)

            ready = {}
            for t0 in range(0, T, 16):
                r = t0 % SEG
                ready.setdefault(max(r + BURN + 15, SEG + BURN - 1 - r), []).append(t0)
            p1q, p2q = [], []
            for k in range(48, NSTEP):
                emit_step(k, pumps=(k <= 48))
                p1q.extend(ready.get(k, []))
                if k < NSTEP - 1:
                    # pace part-1 (Pool+ACT only, off the chain's DVE/PE path)
                    # 2/step; part-2 (PE transposes) strictly post-chain
                    for _ in range(min(2, len(p1q))):
                        t0 = p1q.pop(0)
                        emit_combine_p1(t0, on_pool=True)
                        p2q.append(t0)
            # scheduler fence: keep the tail's DVE/PE work out of the
            # latency-critical chain (no runtime sync cost)
            tc.no_sync_barrier()
            # post-chain: remaining part-1 on DVE (idle now), then part-2
            for t0 in p1q:
                emit_combine_p1(t0, on_pool=False)
                p2q.append(t0)
            for t0 in p2q:
                emit_combine_p2(t0)

    _strip_waits(nc)
    return nc


def _strip_waits(nc):
    """Reduce every instruction to <=1 sync wait (walrus limit), using only
    drops that hardware ordering or this kernel's structure guarantees:
    - duplicate-sem waits merged to the max value (always sound);
    - PE->PE waits on PE instructions: PE completions are pc-monotone;
    - ACT evictions' DVE waits: bounding-box WAR artifacts vs chain muls
      reading strictly different QBUF/Q3 stripe columns;
    - DMA-DMA waits on output DMAs: disjoint OUT regions (box artifacts);
    - kernel-tail drains keep the out-DMA wait.
    """
    import concourse.mybir as mybir

    own_sem = {"Pool": "Pool_", "DVE": "DVE_", "Activation": "Activation_",
               "PE": "PE_"}
    warn = []
    for f in nc.m.functions:
        for bb in f.blocks:
            for inst in bb.instructions:
                si = inst.sync_info
                if si is None or len(si.on_wait) == 0:
                    continue
                tn = type(inst).__name__
                eng = str(inst.engine).split(".")[-1]
                if len(si.on_wait) == 1:
                    # single-wait fast path: only the Pool-on-DVE artifact
                    # drop applies (see below)
                    if (eng == "DVE" and tn != "InstDMACopy"
                            and si.on_wait[0].ant_name.startswith("Pool_")):
                        inst.sync_info = mybir.SyncInfo(
                            on_wait=[], on_update=list(si.on_update))
                    continue
                best = {}
                for x in si.on_wait:
                    if x.ant_name not in best or x.wait_value > best[x.ant_name].wait_value:
                        best[x.ant_name] = x
                w = list(best.values())

                def setw(w2):
                    inst.sync_info = mybir.SyncInfo(
                        on_wait=w2, on_update=list(si.on_update))

                # own-engine sem waits: engines execute in issue order
                own = own_sem.get(eng)
                if own and len(w) > 1 and tn != "InstDMACopy":
                    w = [x for x in w if not x.ant_name.startswith(own)] or w[:1]
                # no DVE instruction in this kernel reads Pool-written data:
                # Pool->DVE waits are qstore bounding-box WAR artifacts
                # (combine reads of old steps vs chain writes of new steps)
                if eng == "DVE" and tn != "InstDMACopy":
                    w = [x for x in w if not x.ant_name.startswith("Pool_")]
                if len(w) <= 1:
                    setw(w)
                    continue
                if tn in ("InstMatmult", "InstLdweights"):
                    w = [x for x in w if not x.ant_name.startswith("PE_")]
                elif tn == "InstActivation":
                    pe = [x for x in w if x.ant_name.startswith("PE_")]
                    if pe:
                        w = pe
                elif tn == "InstDMACopy":
                    w = [x for x in w if not (
                        x.ant_name.startswith("DMASW")
                        or x.ant_name.startswith("DMAHW"))]
                elif tn == "InstDrain":
                    w.sort(key=lambda x: 0 if x.ant_name.startswith("DMA") else 1)
                    w = w[:1]
                if len(w) > 1:
                    warn.append((tn, str(inst.engine), [x.ant_name for x in w]))
                    rank = {"PE": 0, "Ac": 1, "DV": 2, "Po": 3}
                    w.sort(key=lambda x: rank.get(x.ant_name[:2], 4))
                    w = w[:1]
                setw(w)
    if warn:
        from collections import Counter
        cnt = Counter((t, e, tuple(ws)) for t, e, ws in warn)
        for k, v in cnt.items():
            print(f"WARN multi-wait fallback x{v}: {k}")


_SLICE_IDX = None


def _slice_indices():
    global _SLICE_IDX
    if _SLICE_IDX is not None:
        return _SLICE_IDX
    out = []
    for i in range(NSLICE):
        ks = _slice_ks(i)
        ts = np.zeros(SCOLS, np.int64)
        bs = np.zeros(SCOLS, np.int64)
        for d in range(2):
            for dk in range(4):
                k = ks[dk]
                s = _sF(k) if d == 0 else _sB(k)
                for g in range(8):
                    for b in range(8):
                        j = d * 256 + dk * 64 + g * 8 + b
                        ts[j] = g * SEG + s
                        bs[j] = b
        out.append((ts, bs))
    _SLICE_IDX = out
    return out


def host_inputs(X, kernel, chain_kernel, bias, left_boundary, right_boundary):
    X = np.asarray(X, np.float32)
    W = np.asarray(kernel, np.float32)
    C = np.asarray(chain_kernel, np.float32)
    bias = np.asarray(bias, np.float32)
    lb = np.asarray(left_boundary, np.float32)
    rb = np.asarray(right_boundary, np.float32)

    EW16 = np.exp(-C.astype(np.float64) - CSCALE).astype(np.float16)
    W16 = np.ascontiguousarray(
        W.astype(np.float16).reshape(16, 128, 128).transpose(1, 0, 2)
    ).reshape(128, 16 * 128)
    nb0, nb1, nb2 = -(bias + lb), -bias, -(bias + rb)
    NB = np.stack([nb0, nb1, nb2, 3 * nb0, 3 * nb1, 3 * nb2], axis=1).astype(np.float32)
    IDN = np.eye(F, dtype=np.float32)

    X16 = X.astype(np.float16)
    idx = _slice_indices()
    in_maps = []
    for c in range(NCORES):
        Xc = X16[c * BL:(c + 1) * BL]  # (8, 512, 2048)
        xts = np.empty((NSLICE, 128, 16 * SCOLS), np.float16)
        for i in range(NSLICE):
            ts, bs = idx[i]
            cols = Xc[bs, ts, :]  # (512, 2048)
            xts[i] = np.ascontiguousarray(
                cols.T.reshape(16, 128, SCOLS).transpose(1, 0, 2)
            ).reshape(128, 16 * SCOLS)
        in_maps.append({
            "xt": xts, "w": W16, "ew": EW16, "nb": NB, "idn": IDN,
        })
    return in_maps


_NC_CACHE = None


def kernel(X, kernel, chain_kernel, bias, left_boundary, right_boundary):
    global _NC_CACHE
    from concourse.bass_utils import run_bass_kernel_spmd

    if _NC_CACHE is None:
        _NC_CACHE = build_nc()
    nc = _NC_CACHE
    in_maps = host_inputs(X, kernel, chain_kernel, bias, left_boundary, right_boundary)
    res = run_bass_kernel_spmd(nc, in_maps, list(range(NCORES)))
    return postprocess(res)


def postprocess(res):
    # device returns pre-softmax margins (fp32); normalize on host
    m = np.concatenate(
        [np.asarray(res.results[c]["out"], np.float32) for c in range(NCORES)],
        axis=0)
    m -= m.max(-1, keepdims=True)
    np.exp(m, out=m)
    m /= m.sum(-1, keepdims=True)
    return m
